# revision 33
# baseline (speedup 1.0000x reference)
"""DisentangleMultiHeadedAttention — Trainium2 Bass kernel (8 NeuronCores).

Contract: kernel(**inputs) takes the FULL unsharded inputs of
reference.setup_inputs() and returns the full output tuple
(out_a [4,1024,1024] f32, out_b [4,1024,1024] f32).

Sharding: 8 cores = 4 batches x 2 streams, fully data-parallel (no
collectives); core c computes stream (c//4), batch (c%4). The
dual-stream score fusion qa.(ka+kb) is computed by summing the two K
projections once per core, halving score matmuls.

Per-core kernel (software-pipelined): all matmul operands bf16 (fp32
PSUM accumulation). V projection runs first (wv resident); then a
band loop interleaves the fused Ka+Kb (one 16-matmul PSUM chain) and
Q projections of output band ob with the attention pipeline
(scores->exp->AV->normalize) of that band's two heads, so the ~110us
of Exp work on the Scalar engine overlaps projection matmuls instead
of serializing after them. Softmax denominators come from a 64-wide
ones block appended to each head's V (the AV matmul produces them in
PSUM partitions 64..127); 1/denominator is computed as exp(-ln(d))
on the Scalar engine (DVE InstReciprocal is ~6.6ns/elem and
head-of-line-blocks band-critical writebacks on the in-order Vector
engine). The key-padding mask is applied by zeroing masked keys'
rows of the augmented V (equivalent to the reference's -1e9
masking). Softmax max-subtraction is skipped (scores ~N(0,1)).

DMA shaping: every DRAM tensor is laid out host-side so each transfer
is one dma_start with large per-partition-contiguous descriptors
(16KB whole-tensor loads for x streams; 4KB 2-band granules for the
projection weights, band-major; st-major xv / sh-major wv so the V
phase can start after a couple of small granules). A queue's
in-flight transfers progress proportionally rather than FIFO, so
transfers are issued in staged waves gated by WAW dummy-write
dependencies on early V compute: wave 0 covers the start of the V
phase, wave 1b the rest of V, wave 2 the k/q x-streams; weight
granules stream two bands ahead inside the band loop.
"""
import math
import numpy as np
import concourse.bass as bass
import concourse.mybir as mybir
import concourse.tile as tile


MAX_WAITS = 1


def _split_excess_waits(nc):
    ctr = [0]

    def mknop(engine, chunk):
        ctr[0] += 1
        nop = mybir.InstNoOp(name=f"waitfix-nop-{ctr[0]}", ins=[], outs=[])
        nop.engine = engine
        nop.sync_info = mybir.SyncInfo(on_wait=chunk, on_update=[])
        return nop

    for f in nc.m.functions:
        for b in f.blocks:
            insts = b.instructions
            new = []
            changed = False
            for i in insts:
                si = i.sync_info
                if si is not None and len(si.on_wait) > MAX_WAITS:
                    waits = list(si.on_wait)
                    while len(waits) > MAX_WAITS:
                        chunk, waits = waits[:MAX_WAITS], waits[MAX_WAITS:]
                        new.append(mknop(i.engine, chunk))
                    i.sync_info = mybir.SyncInfo(
                        on_wait=waits, on_update=list(si.on_update)
                    )
                    changed = True
                new.append(i)
            if changed:
                b.instructions = new


DT = mybir.dt
B, S, HID, HEADS = 4, 1024, 1024, 16
DH = HID // HEADS          # 64
NO = 64                    # ones-block width (denominator replicas)
P = 128
NB = HID // P              # 8
NH = S // 512              # 2
SCALE = 1.0 / math.sqrt(2 * DH)


def build_nc():
    bf16 = DT.bfloat16
    f32 = DT.float32
    nc = bass.Bass()

    dp = nc.declare_dram_parameter
    # x streams: [feature-part, feature-band, s]
    xq_t = dp("xq_t", [P, NB, S], bf16, isOutput=False)
    xka_t = dp("xka_t", [P, NB, S], bf16, isOutput=False)
    xkb_t = dp("xkb_t", [P, NB, S], bf16, isOutput=False)
    # xv: st-major so the V phase starts after one 256KB granule
    xv_t = dp("xv_t", [P, NB, NB, P], bf16, isOutput=False)
    # projection weights: out-band-major 2-band granules
    wq_t = dp("wq_t", [P, NB, NB, P], bf16, isOutput=False)
    wka_t = dp("wka_t", [P, NB, NB, P], bf16, isOutput=False)
    wkb_t = dp("wkb_t", [P, NB, NB, P], bf16, isOutput=False)
    # wv: sh-major halves
    wv_t = dp("wv_t", [P, NH, NB, 512], bf16, isOutput=False)
    wo_t = dp("wo_t", [P, NB, NB, P], bf16, isOutput=False)
    bq = dp("bq", [P, NB], f32, isOutput=False)
    bk = dp("bk", [P, NB], f32, isOutput=False)
    bo = dp("bo", [P, NB], f32, isOutput=False)
    m01 = dp("m01", [P, NB], f32, isOutput=False)
    ones = dp("ones", [P, NO], bf16, isOutput=False)
    outT = dp("outT", [HID, S], f32, isOutput=True)

    with tile.TileContext(nc) as tc:
        with (
            tc.tile_pool(name="persist", bufs=1) as persist,
            tc.tile_pool(name="small", bufs=1) as small,
        ):
            qT = persist.tile([P, NB, S], bf16, tag="qT")
            kT = persist.tile([P, NB, S], bf16, tag="kT")
            hT = persist.tile([P, NB, S], bf16, tag="hT")
            v_aug = persist.tile([P, NB, HEADS, DH + NO], bf16, tag="va")
            bq_sb = small.tile([P, NB], f32, tag="bq")
            bk_sb = small.tile([P, NB], f32, tag="bk")
            bo_sb = small.tile([P, NB], f32, tag="bo")
            m01_sb = small.tile([P, NB], f32, tag="m01")
            on_sb = small.tile([P, NO], bf16, tag="on")


            with (
                tc.tile_pool(name="xres", bufs=1) as xres,
                tc.tile_pool(name="wstr", bufs=2) as wstr,
                tc.tile_pool(name="psq", bufs=2, space="PSUM") as psq,
            ):
                xq = xres.tile([P, NB, S], bf16, tag="xq")
                xka = xres.tile([P, NB, S], bf16, tag="xka")
                xkb = xres.tile([P, NB, S], bf16, tag="xkb")

                def w_granule(g, eng=None):
                    """2-band weight granule (bands 2g, 2g+1) for ka/kb/q."""
                    wkag = wstr.tile([P, 2, NB, P], bf16, tag="wka")
                    wkbg = wstr.tile([P, 2, NB, P], bf16, tag="wkb")
                    wqg = wstr.tile([P, 2, NB, P], bf16, tag="wq")
                    gs = slice(2 * g, 2 * g + 2)
                    (eng or nc.sync).dma_start(wkag[:], wka_t[:, gs, :, :])
                    (eng or nc.scalar).dma_start(wkbg[:], wkb_t[:, gs, :, :])
                    (eng or nc.gpsimd).dma_start(wqg[:], wq_t[:, gs, :, :])
                    return wkag, wkbg, wqg

                # ---------------- V projection (untransposed) ------------
                with tc.tile_pool(name="vx", bufs=1) as vx:
                    xv = vx.tile([P, NB, NB, P], bf16, tag="xv")
                    wv = vx.tile([P, NH, NB, 512], bf16, tag="wv")
                    # strict need-order, round-robin over the 3 DMA-capable
                    # queues: tiny constants (their DVE/act consumers are
                    # scheduled early — late arrival head-of-line-blocks
                    # the in-order engines), wv-sh0 + first xv granules,
                    # rest of xv, wv-sh1, first weight granule, k/q
                    # x-streams.
                    nc.sync.dma_start(m01_sb[:], m01[:])
                    nc.scalar.dma_start(on_sb[:], ones[:])
                    nc.gpsimd.dma_start(bk_sb[:], bk[:])
                    nc.sync.dma_start(bq_sb[:], bq[:])
                    nc.scalar.dma_start(bo_sb[:], bo[:])
                    nc.sync.dma_start(wv[:, 0, 0:4], wv_t[:, 0, 0:4])
                    nc.scalar.dma_start(wv[:, 0, 4:8], wv_t[:, 0, 4:8])
                    eng3 = (nc.gpsimd, nc.sync, nc.scalar)
                    for st in range(4):
                        eng3[st % 3].dma_start(xv[:, st], xv_t[:, st])
                    wg = w_granule(0, eng=nc.scalar)
                    # k/q x-streams (whole-tensor, 16KB descriptors), in
                    # P1 use order ka, kb, q
                    wg2 = w_granule(1)

                    # ones block of v_aug (masked)
                    for st in range(NB):
                        nc.vector.tensor_scalar_mul(
                            v_aug[:, st, :, DH:DH + NO],
                            on_sb.unsqueeze(1).to_broadcast([P, HEADS, NO]),
                            m01_sb[:, st:st + 1],
                        )

                    for sh in range(NH):
                        for st in range(NB):
                            psv = psq.tile([P, 512], f32, tag="ps")
                            for i in range(NB):
                                nc.tensor.matmul(
                                    psv[:],
                                    xv[:, st, i, :],
                                    wv[:, sh, i, :],
                                    start=(i == 0), stop=(i == NB - 1),
                                )
                            nc.vector.tensor_scalar_mul(
                                v_aug[:, st, 8 * sh:8 * (sh + 1), 0:DH],
                                psv[:].rearrange("p (h d) -> p h d", d=DH),
                                m01_sb[:, st:st + 1],
                            )
                            if sh == 0 and st == 0:
                                # staged DMA waves, gated by real WAW deps
                                # on early V compute (a queue's in-flight
                                # transfers progress proportionally, not
                                # FIFO, so later waves must not be
                                # enqueued early).  Wave 1b: rest of the
                                # V-phase inputs; wave 2: k/q x-streams.
                                for st2 in range(4, NB):
                                    nc.scalar.copy(
                                        xv[0:1, st2, 0, 0:1],
                                        v_aug[0:1, 0, 0, 0:1])
                                    eng3[st2 % 3].dma_start(
                                        xv[:, st2], xv_t[:, st2])
                                nc.scalar.copy(
                                    wv[0:1, 1, 0, 0:1],
                                    v_aug[0:1, 0, 0, 0:1])
                                nc.scalar.copy(
                                    wv[0:1, 1, 4, 0:1],
                                    v_aug[0:1, 0, 0, 0:1])
                                nc.gpsimd.dma_start(
                                    wv[:, 1, 0:4], wv_t[:, 1, 0:4])
                                nc.sync.dma_start(
                                    wv[:, 1, 4:8], wv_t[:, 1, 4:8])
                            if sh == 0 and st == 2:
                                for xt in (xka, xkb, xq):
                                    nc.scalar.copy(
                                        xt[0:1, 0, 0:1],
                                        v_aug[0:1, 1, 0, 0:1])
                                nc.scalar.dma_start(xka[:], xka_t[:])
                                nc.scalar.dma_start(xkb[:], xkb_t[:])
                                nc.scalar.dma_start(xq[:], xq_t[:])

                # ---------------- band loop: P1(ob) then P2(heads of ob) --
                with (
                    tc.tile_pool(name="wop", bufs=1) as wop,
                    tc.tile_pool(name="eTp", bufs=3) as epool,
                    tc.tile_pool(name="rcp", bufs=1) as rcpool,
                    tc.tile_pool(name="otp", bufs=2) as otpool,
                    tc.tile_pool(name="pss", bufs=2, space="PSUM") as pss,
                    tc.tile_pool(name="psa", bufs=2, space="PSUM") as psa,
                ):
                    wo_sb = wop.tile([P, NB, NB, P], bf16, tag="wo")
                    wgs = [wg, wg2]
                    for ob in range(NB):
                        g2 = ob % 2
                        if ob == 4:
                            # prefetch output-projection weights during
                            # late attention
                            nc.sync.dma_start(wo_sb[:], wo_t[:])
                        wkag, wkbg, wqg = wgs[ob // 2]

                        # P1 for band ob: fused ka+kb accumulation (one
                        # 16-matmul PSUM chain, one writeback), then q.
                        # K first so the kT writeback latency hides under
                        # the q matmuls before P2's score matmuls need kT.
                        for sh in range(NH):
                            sq = slice(sh * 512, (sh + 1) * 512)
                            ps = psq.tile([P, 512], f32, tag="ps")
                            for i in range(NB):
                                nc.tensor.matmul(
                                    ps[:], wkag[:, g2, i, :], xka[:, i, sq],
                                    start=(i == 0), stop=False,
                                )
                            for i in range(NB):
                                nc.tensor.matmul(
                                    ps[:], wkbg[:, g2, i, :], xkb[:, i, sq],
                                    start=False, stop=(i == NB - 1),
                                )
                            nc.vector.tensor_scalar_add(
                                kT[:, ob, sq], ps[:], bk_sb[:, ob:ob + 1],
                            )
                        for sh in range(NH):
                            sq = slice(sh * 512, (sh + 1) * 512)
                            ps = psq.tile([P, 512], f32, tag="ps")
                            for i in range(NB):
                                nc.tensor.matmul(
                                    ps[:], wqg[:, g2, i, :], xq[:, i, sq],
                                    start=(i == 0), stop=(i == NB - 1),
                                )
                            nc.vector.tensor_scalar_add(
                                qT[:, ob, sq], ps[:], bq_sb[:, ob:ob + 1],
                            )

                        if g2 == 1 and ob // 2 + 2 <= NB // 2 - 1:
                            # the granule ring slot of granule ob//2 frees
                            # once this band's P1 is emitted; start the
                            # next granule's DMA now so it lands before
                            # the band-end act-bound window
                            wgs.append(w_granule(ob // 2 + 2))

                        # P2: attention for heads 2*ob, 2*ob+1.  Scores
                        # stream the full 1024-query band per sk-tile (one
                        # matmul per sk-tile instead of two 512-wide ones
                        # — halves the chain-start semaphore waits on the
                        # PSUM ring).
                        for h in (2 * ob, 2 * ob + 1):
                            pt = (h % 2) * DH
                            for sh in range(NH):
                                sq = slice(sh * 512, (sh + 1) * 512)
                                eT = epool.tile([P, NB, 512], bf16, tag="eT")
                                for skp in range(NB // 2):
                                    ps = pss.tile([P, 1024], f32, tag="ps")
                                    for j in range(2):
                                        skt = 2 * skp + j
                                        nc.tensor.matmul(
                                            ps[:, j * 512:(j + 1) * 512],
                                            kT[pt:pt + DH, ob,
                                               skt * P:(skt + 1) * P],
                                            qT[pt:pt + DH, ob, sq],
                                            start=True, stop=True,
                                        )
                                    nc.scalar.activation(
                                        eT[:, 2 * skp:2 * skp + 2, :],
                                        ps[:].rearrange(
                                            "p (j n) -> p j n", n=512),
                                        mybir.ActivationFunctionType.Exp,
                                        scale=SCALE,
                                    )
                                pa = psa.tile([P, 512], f32, tag="pa")
                                for skt in range(NB):
                                    nc.tensor.matmul(
                                        pa[0:DH + NO, :],
                                        v_aug[:, skt, h, :],
                                        eT[:, skt, :],
                                        start=(skt == 0), stop=(skt == NB - 1),
                                    )
                                # 1/d as exp(-ln(d)) on the Scalar engine:
                                # DVE's InstReciprocal is ~6.6ns/elem and
                                # head-of-line-blocks the in-order Vector
                                # engine ahead of band-critical writebacks
                                rc = rcpool.tile([NO, 512], f32, tag="rc")
                                nc.scalar.activation(
                                    pa[DH:DH + NO, :], pa[DH:DH + NO, :],
                                    mybir.ActivationFunctionType.Ln,
                                )
                                nc.scalar.activation(
                                    rc[:], pa[DH:DH + NO, :],
                                    mybir.ActivationFunctionType.Exp,
                                    scale=-1.0,
                                )
                                nc.vector.tensor_mul(
                                    hT[pt:pt + DH, ob, sq],
                                    pa[0:DH, :], rc[0:DH, :],
                                )

                    # ---------------- output projection -------------------
                    # o2b-outer so both sq-halves of an output band merge
                    # into one 512KB DMA (4KB descriptors, half the tail
                    # descriptor count)
                    for o2b in range(NB):
                        ob_t = otpool.tile([P, S], f32, tag="ot")
                        for sh in range(NH):
                            sq = slice(sh * 512, (sh + 1) * 512)
                            ps = psq.tile([P, 512], f32, tag="ps")
                            for i in range(NB):
                                nc.tensor.matmul(
                                    ps[:],
                                    wo_sb[:, o2b, i, :],
                                    hT[:, i, sq],
                                    start=(i == 0), stop=(i == NB - 1),
                                )
                            nc.vector.tensor_scalar_add(
                                ob_t[:, sq], ps[:], bo_sb[:, o2b:o2b + 1]
                            )
                        nc.sync.dma_start(
                            outT[o2b * P:(o2b + 1) * P, :],
                            ob_t[:],
                        )
    return nc


def _band(a_t):
    """[1024, N] -> band-major [128, 8, N]."""
    return np.ascontiguousarray(
        a_t.reshape(NB, P, a_t.shape[1]).transpose(1, 0, 2)
    )


def _obmajor(a_t):
    """[1024, 1024] (in, out) -> [128, ob, i, 128] out-band-major."""
    b = _band(a_t)  # [P, i, 1024]
    return np.ascontiguousarray(
        b.reshape(P, NB, NB, P).transpose(0, 2, 1, 3)
    )


def host_prepare(q_a, k_a, v_a, q_b, k_b, v_b, mask, Wa, ba, Wb, bb,
                 Wo_a, bo_a, Wo_b, bo_b):
    """Per-core input maps. Core c = stream (c // 4), batch (c % 4)."""
    import ml_dtypes
    f32 = np.float32
    bf = ml_dtypes.bfloat16
    tb = lambda a: _band(np.asarray(a, f32).T.astype(bf))
    wb_ = lambda a: _obmajor(np.asarray(a, f32).T.astype(bf))
    col = lambda v: np.ascontiguousarray(np.asarray(v, f32).reshape(NB, P).T)

    def vst(a):
        # x_v transposed banded [P, i, S] -> st-major [P, st, i, 128]
        b = tb(a)
        return np.ascontiguousarray(
            b.reshape(P, NB, NB, P).transpose(0, 2, 1, 3))

    def wsh(a):
        # wv banded [P, i, HID] -> sh-major [P, sh, i, 512]
        b = _band(np.asarray(a, f32).T.astype(bf))
        return np.ascontiguousarray(
            b.reshape(P, NB, NH, 512).transpose(0, 2, 1, 3))

    wq = {0: wb_(Wa[0]), 1: wb_(Wb[0])}
    wka, wkb = wb_(Wa[1]), wb_(Wb[1])
    wv = {0: wsh(Wa[2]), 1: wsh(Wb[2])}
    wo = {0: wb_(Wo_a), 1: wb_(Wo_b)}
    bqc = {0: col(ba[0]), 1: col(bb[0])}
    bkc = col(np.asarray(ba[1], f32) + np.asarray(bb[1], f32))
    boc = {0: col(bo_a), 1: col(bo_b)}
    ones = np.ones((P, NO), bf)
    mask = np.asarray(mask)
    q = {0: q_a, 1: q_b}
    v = {0: v_a, 1: v_b}

    in_maps = []
    for c in range(8):
        s, b = c // 4, c % 4
        mb = (mask[b] != 0).astype(f32)
        in_maps.append({
            "xq_t": tb(q[s][b]), "xka_t": tb(k_a[b]),
            "xkb_t": tb(k_b[b]),
            "xv_t": vst(v[s][b]),
            "wq_t": wq[s], "wka_t": wka, "wkb_t": wkb, "wv_t": wv[s],
            "wo_t": wo[s],
            "bq": bqc[s], "bk": bkc, "bo": boc[s],
            "m01": col(mb), "ones": ones,
        })
    return in_maps


def assemble(results):
    out_a = np.stack([results[b]["outT"].T for b in range(4)])
    out_b = np.stack([results[4 + b]["outT"].T for b in range(4)])
    return out_a, out_b


_CACHE = {}


def _get_nc():
    if "nc" not in _CACHE:
        nc = build_nc()
        _split_excess_waits(nc)
        _CACHE["nc"] = nc
    return _CACHE["nc"]


def kernel(**inputs):
    from concourse.bass_utils import run_bass_kernel_spmd

    nc = _get_nc()
    in_maps = host_prepare(**{k: np.asarray(v) for k, v in inputs.items()})
    res = run_bass_kernel_spmd(nc, in_maps, list(range(8)))
    return assemble(res.results)
